# revision 19
# baseline (speedup 1.0000x reference)
"""CFM contrastive loss on 8 TRN2 NeuronCores — transposed j-shard design.

loss = -mean(diag(log_softmax(logits))),  logits[i,j] = 2*z1_i.z2_j - |z1_i|^2 - |z2_j|^2

With t[i,j] = 2*z1_i.z2_j - |z2_j|^2 the loss is mean_i(log(sum_j exp(t_ij)) - t_ii);
the |z1_i|^2 term cancels.  t spans ~[-317, +54] but per-row only terms within
~20 of the row max matter, and row maxes span ~[-70, +54]: with a global shift
C=30, exp(t+C) fits bf16 (max e^84; terms that flush to zero are >=47 below the
weakest row max — negligible).

Sharding: z2 rows (j) are split across 8 cores (1024 each = 8 partition tiles);
every core reads all of z1 as the matmul moving operand.  Layout is transposed
vs the usual: g^T[j, i] = lhsT(z2-tile).T @ (2*z1).T, so j sits on PSUM
partitions and the per-j offset C - |z2_j|^2 rides the ACT activation's
per-partition bias — no separate w-multiply pass (the old STT ran at 1x DVE
mode, ~2.3us/tile).

Per (i-chunk, j-tile) step of [128, 2048]:
  - PE: matmul into PSUM (K=128, one 2048-wide or four 512-wide)
  - exp path, one of:
      ACT: e = exp(psum + bias_j)  -> bf16  (~1.97us/tile)
      DVE Schraudolph: i16 = rne_sat(psum*(128/ln2) + b2_j) via tensor_scalar
        (f32->i16 saturates: underflow -> -32768 = bf16 -0.0), bitcast = bf16
        2^x approx (~2.29us/tile, relerr ~1e-5 on the loss)
  - DVE: acc[:, ichunk] += e (bf16 tensor_tensor, 2x DVE mode ~1.2us/tile);
    jt==0 writes acc directly from ACT (or a 4x-mode copy for Schraudolph)
Host: partition-sum the 8 acc[128, 8192] outputs in f64, then
loss = mean(log(rows) - C - tdiag) + the cheap O(N*D) diagonal term.
"""

import numpy as np
import ml_dtypes

N, D = 8192, 128
NCORES = 8
JSHARD = N // NCORES         # 1024 z2 rows per core
JTILES = JSHARD // 128       # 8 partition tiles
ICHUNK = 2048                # PSUM chunk (4 banks of 512 fp32)
NIC = N // ICHUNK            # 4 i-chunks
CSHIFT = 30.0                # global shift: e = exp(t + C)
SCHRA_A = 128.0 / np.log(2.0)      # Schraudolph slope (bf16 bit domain)
SCHRA_B = 16256.0 - 7.0            # 127*128 + beta, beta=-7 calibrated
WIDE_MM = False              # 2048-wide matmul fails walrus ISA check: 4x512
NOLOAD_MM = False            # ldweights strip only pays with jt-outer ordering
N_SCH = 7                    # of the 32 steps, how many use DVE Schraudolph
BF16 = ml_dtypes.bfloat16

_NC_CACHE = None


def _sch_steps():
    """Which (ic, jt) steps use the DVE Schraudolph path: spread N_SCH of the
    32 steps evenly, never jt==0 (jt==0 is ACT's free direct-write)."""
    steps = [(ic, jt) for ic in range(NIC) for jt in range(JTILES)]
    cand = [s for s in steps if s[1] != 0]
    if N_SCH == 0:
        return set()
    stride = len(cand) / N_SCH
    return {cand[min(int(k * stride), len(cand) - 1)] for k in range(N_SCH)}


def _strip_redundant_ldweights(nc):
    """Drop back-to-back InstLdweights that reload the identical weights AP.

    Tile legalization inserts one InstLdweights per InstMatmult regardless of
    whether the stationary operand changed (~100ns each on the PE queue, 128
    per core here).  With the jt-outer loop, 16 consecutive matmuls share one
    weights tile, so all but the first load of each run are pure overhead.
    Only waitless/updateless loads are removed — semaphore waits that
    move_matmul_waits_to_ldweights parked on a load must stay."""
    removed = 0
    for f in nc.m.functions:
        for bb in f.blocks:
            insts = list(bb.instructions)
            keep = []
            last_sig = None
            for inst in insts:
                nm = type(inst).__name__
                if nm == "InstLdweights":
                    sig = repr(inst.ins)
                    si = inst.sync_info
                    waitless = si is None or (
                        len(si.on_wait) == 0 and len(si.on_update) == 0
                    )
                    if sig == last_sig and waitless:
                        removed += 1
                        continue
                    last_sig = sig
                keep.append(inst)
            if removed:
                while len(bb.instructions):
                    bb.instructions.pop()
                for inst in keep:
                    bb.instructions.append(inst)
    return removed


def _build_nc():
    import concourse.mybir as mybir
    import concourse.tile as tile
    from concourse import bacc

    nc = bacc.Bacc(None, target_bir_lowering=False)

    z1t2 = nc.dram_tensor("z1t2", [128, N], mybir.dt.bfloat16, kind="ExternalInput")
    z2t = nc.dram_tensor("z2t", [128, JSHARD], mybir.dt.bfloat16, kind="ExternalInput")
    biasd = nc.dram_tensor("biasd", [128, JTILES], mybir.dt.float32, kind="ExternalInput")
    b2d = nc.dram_tensor("b2d", [128, JTILES], mybir.dt.float32, kind="ExternalInput")
    accd = nc.dram_tensor("accd", [128, N], mybir.dt.bfloat16, kind="ExternalOutput")

    EXP = mybir.ActivationFunctionType.Exp
    ADD = mybir.AluOpType.add
    MULT = mybir.AluOpType.mult
    sch = _sch_steps()

    with tile.TileContext(nc) as tc:
        with (
            tc.tile_pool(name="const", bufs=1) as cpool,
            tc.tile_pool(name="esc", bufs=3) as epool,
            tc.tile_pool(name="psum", bufs=2, space="PSUM") as ppool,
        ):
            z1_sb = cpool.tile([128, N], mybir.dt.bfloat16)
            z2_sb = cpool.tile([128, JSHARD], mybir.dt.bfloat16)
            bias_sb = cpool.tile([128, JTILES], mybir.dt.float32)
            b2_sb = cpool.tile([128, JTILES], mybir.dt.float32)
            acc_sb = cpool.tile([128, N], mybir.dt.bfloat16)

            # DMA order: what the first matmuls need lands first; the first
            # 512 z1 columns ship alone so matmul 0 starts ~1us sooner
            nc.sync.dma_start(z2_sb[:], z2t[:])
            nc.sync.dma_start(z1_sb[:, 0:512], z1t2[:, 0:512])
            nc.sync.dma_start(z1_sb[:, 512:ICHUNK], z1t2[:, 512:ICHUNK])
            nc.sync.dma_start(bias_sb[:], biasd[:])
            nc.sync.dma_start(b2_sb[:], b2d[:])
            for ic in range(1, NIC):
                sl = slice(ic * ICHUNK, (ic + 1) * ICHUNK)
                nc.sync.dma_start(z1_sb[:, sl], z1t2[:, sl])

            # The TT merge into acc trails its producer by one step so the
            # psum-freeing consumer (ACT exp or DVE affine) issues first on
            # the DVE queue; flushed before each acc DMA so the read-after-
            # write dependency stays intact.
            pending_tt = None

            def flush_tt():
                nonlocal pending_tt
                if pending_tt is not None:
                    src, dsl = pending_tt
                    nc.vector.tensor_tensor(
                        out=acc_sb[:, dsl], in0=acc_sb[:, dsl], in1=src, op=ADD
                    )
                    pending_tt = None

            for ic in range(NIC):
                sl = slice(ic * ICHUNK, (ic + 1) * ICHUNK)
                for jt in range(JTILES):
                    lhsT = z2_sb[:, jt * 128 : (jt + 1) * 128]
                    ps = ppool.tile([128, ICHUNK], mybir.dt.float32)
                    for b in range(4):
                        j0 = ic * ICHUNK + b * 512
                        nc.tensor.matmul(
                            ps[:, b * 512 : (b + 1) * 512],
                            lhsT,
                            z1_sb[:, j0 : j0 + 512],
                            start=True,
                            stop=True,
                        )
                    if (ic, jt) in sch:
                        # uint16 out: f32->u16 saturates negatives to 0 =
                        # bf16 +0.0, giving exp underflow for free (i16 would
                        # leave small negatives as NaN-pattern bit garbage)
                        it = epool.tile([128, ICHUNK], mybir.dt.uint16)
                        nc.vector.tensor_scalar(
                            out=it[:], in0=ps[:],
                            scalar1=SCHRA_A, scalar2=b2_sb[:, jt : jt + 1],
                            op0=MULT, op1=ADD,
                        )
                        flush_tt()
                        pending_tt = (it[:].bitcast(mybir.dt.bfloat16), sl)
                    elif jt == 0:
                        nc.scalar.activation(
                            acc_sb[:, sl], ps[:], EXP, bias=bias_sb[:, jt : jt + 1]
                        )
                        flush_tt()
                    else:
                        e = epool.tile([128, ICHUNK], mybir.dt.bfloat16)
                        nc.scalar.activation(
                            e[:], ps[:], EXP, bias=bias_sb[:, jt : jt + 1]
                        )
                        flush_tt()
                        pending_tt = (e[:], sl)
                flush_tt()
                nc.sync.dma_start(accd[:, sl], acc_sb[:, sl])

    nc.compile()
    if NOLOAD_MM:
        n = _strip_redundant_ldweights(nc)
        assert n > 60, f"ldweights strip removed only {n}"
    return nc


def _get_nc():
    global _NC_CACHE
    if _NC_CACHE is None:
        _NC_CACHE = _build_nc()
    return _NC_CACHE


def _prep_inputs(z1, z2):
    z1 = np.asarray(z1, dtype=np.float32)
    z2 = np.asarray(z2, dtype=np.float32)
    z1t2 = np.ascontiguousarray((2.0 * z1.astype(np.float64)).astype(BF16).T)
    z2b = z2.astype(BF16)
    sq2 = (z2b.astype(np.float64) ** 2).sum(axis=-1)  # from the bf16 values
    bias_full = (CSHIFT - sq2).astype(np.float32)     # [N]
    b2_full = (SCHRA_A * bias_full.astype(np.float64) + SCHRA_B).astype(np.float32)
    in_maps = []
    for c in range(NCORES):
        jsl = slice(c * JSHARD, (c + 1) * JSHARD)
        z2t = np.ascontiguousarray(z2b[jsl].T)  # [128, JSHARD]
        bias = np.ascontiguousarray(
            bias_full[jsl].reshape(JTILES, 128).T  # [128, JTILES]
        )
        b2 = np.ascontiguousarray(b2_full[jsl].reshape(JTILES, 128).T)
        in_maps.append({"z1t2": z1t2, "z2t": z2t, "biasd": bias, "b2d": b2})
    return in_maps


def _finish(z1, z2, res_list):
    rows = np.zeros(N, np.float64)
    for r in res_list:
        rows += np.asarray(r["accd"], np.float64).sum(axis=0)
    z1 = np.asarray(z1, dtype=np.float64)
    z2 = np.asarray(z2, dtype=np.float64)
    sq2 = (z2.astype(BF16).astype(np.float64) ** 2).sum(axis=-1)
    tdiag = 2.0 * (z1 * z2).sum(axis=-1) - sq2
    loss = np.mean(np.log(rows) - CSHIFT - tdiag)
    return np.asarray(loss, dtype=np.float32)


def _ensure_hook_shim():
    """bass_utils imports antenv.axon_hooks whenever tracing is requested
    (e.g. via a BASS_TRACE env var); this image's antenv lacks that module.
    Provide an inert registry so tracing degrades to a warning instead of an
    ImportError.  A previously installed real shim is left untouched."""
    import sys

    try:
        import antenv.axon_hooks  # noqa: F401
    except ImportError:
        import types

        import antenv

        mod = types.ModuleType("antenv.axon_hooks")
        mod._hook = None
        mod.set_axon_ntff_profile_hook = lambda h: setattr(mod, "_hook", h)
        mod.get_axon_ntff_profile_hook = lambda: mod._hook
        sys.modules["antenv.axon_hooks"] = mod
        antenv.axon_hooks = mod


def _run(z1, z2, **spmd_kwargs):
    _ensure_hook_shim()
    from concourse.bass_utils import run_bass_kernel_spmd

    in_maps = _prep_inputs(z1, z2)
    res = run_bass_kernel_spmd(
        _get_nc(), in_maps, core_ids=list(range(NCORES)), **spmd_kwargs
    )
    return _finish(z1, z2, res.results), res


def kernel(z1, z2):
    loss, _ = _run(z1, z2)
    return loss


# revision 22
# speedup vs baseline: 1.0152x; 1.0152x over previous
"""CFM contrastive loss on 8 TRN2 NeuronCores — transposed j-shard design.

loss = -mean(diag(log_softmax(logits))),  logits[i,j] = 2*z1_i.z2_j - |z1_i|^2 - |z2_j|^2

With t[i,j] = 2*z1_i.z2_j - |z2_j|^2 the loss is mean_i(log(sum_j exp(t_ij)) - t_ii);
the |z1_i|^2 term cancels.  t spans ~[-317, +54] but per-row only terms within
~20 of the row max matter, and row maxes span ~[-70, +54]: with a global shift
C=30, exp(t+C) fits bf16 (max e^84; terms that flush to zero are >=47 below the
weakest row max — negligible).

Sharding: z2 rows (j) are split across 8 cores (1024 each = 8 partition tiles);
every core reads all of z1 as the matmul moving operand.  Layout is transposed
vs the usual: g^T[j, i] = lhsT(z2-tile).T @ (2*z1).T, so j sits on PSUM
partitions and the per-j offset C - |z2_j|^2 rides the ACT activation's
per-partition bias — no separate w-multiply pass (the old STT ran at 1x DVE
mode, ~2.3us/tile).

Per (i-chunk, j-tile) step of [128, 2048] (ic-outer so acc chunks stream out
through the run and z1 i-chunks arrive just-in-time):
  - PE: four 512-wide matmuls into PSUM (K=128; 2048-wide fails the walrus
    ISA check; sustained ~375ns/matmul after the p-state ramp)
  - exp path, one of:
      ACT (25 steps): e = exp(psum + bias_j) -> bf16 (~1.97us/tile)
      DVE Schraudolph (7 steps): u16 = rne(psum*(128/ln2) + b2_j) via
        tensor_scalar; the f32->u16 convert saturates negatives to 0 so
        underflow becomes bf16 +0.0, and the bitcast of the u16 IS the bf16
        2^x approx (~2.3us/tile; beta=-7 calibrated, loss relerr ~1e-5)
  - DVE: acc[:, ichunk] += e (bf16 tensor_tensor in 2x DVE mode ~1.2us/tile);
    jt==0 skips the add: ACT writes the acc slice directly
Host: partition-sum the 8 acc[128, 8192] outputs in f64, then
loss = mean(log(rows) - C - tdiag) with the cheap O(N*D) diagonal term.

Engine busy per core: PE ~51us, ACT ~49us, DVE ~51us; HW exec ~79.5us
(vs 94-111us for the previous STT-based kernel).
"""

import numpy as np
import ml_dtypes

N, D = 8192, 128
NCORES = 8
JSHARD = N // NCORES         # 1024 z2 rows per core
JTILES = JSHARD // 128       # 8 partition tiles
ICHUNK = 2048                # PSUM chunk (4 banks of 512 fp32)
NIC = N // ICHUNK            # 4 i-chunks
CSHIFT = 30.0                # global shift: e = exp(t + C)
SCHRA_A = 128.0 / np.log(2.0)      # Schraudolph slope (bf16 bit domain)
SCHRA_B = 16256.0 - 7.0            # 127*128 + beta, beta=-7 calibrated
WIDE_MM = False              # 2048-wide matmul fails walrus ISA check: 4x512
NOLOAD_MM = False            # ldweights strip only pays with jt-outer ordering
N_SCH = 7                    # of the 32 steps, how many use DVE Schraudolph
BF16 = ml_dtypes.bfloat16

_NC_CACHE = None


def _sch_steps():
    """Which (ic, jt) steps use the DVE Schraudolph path: spread N_SCH of the
    32 steps evenly, never jt==0 (jt==0 is ACT's free direct-write)."""
    steps = [(ic, jt) for ic in range(NIC) for jt in range(JTILES)]
    cand = [s for s in steps if s[1] != 0]
    if N_SCH == 0:
        return set()
    stride = len(cand) / N_SCH
    return {cand[min(int(k * stride), len(cand) - 1)] for k in range(N_SCH)}


def _strip_redundant_ldweights(nc):
    """Drop back-to-back InstLdweights that reload the identical weights AP.

    Tile legalization inserts one InstLdweights per InstMatmult regardless of
    whether the stationary operand changed (~100ns each on the PE queue, 128
    per core here).  With the jt-outer loop, 16 consecutive matmuls share one
    weights tile, so all but the first load of each run are pure overhead.
    Only waitless/updateless loads are removed — semaphore waits that
    move_matmul_waits_to_ldweights parked on a load must stay."""
    removed = 0
    for f in nc.m.functions:
        for bb in f.blocks:
            insts = list(bb.instructions)
            keep = []
            last_sig = None
            for inst in insts:
                nm = type(inst).__name__
                if nm == "InstLdweights":
                    sig = repr(inst.ins)
                    si = inst.sync_info
                    waitless = si is None or (
                        len(si.on_wait) == 0 and len(si.on_update) == 0
                    )
                    if sig == last_sig and waitless:
                        removed += 1
                        continue
                    last_sig = sig
                keep.append(inst)
            if removed:
                while len(bb.instructions):
                    bb.instructions.pop()
                for inst in keep:
                    bb.instructions.append(inst)
    return removed


def _build_nc():
    import concourse.mybir as mybir
    import concourse.tile as tile
    from concourse import bacc

    nc = bacc.Bacc(None, target_bir_lowering=False)

    z1t2 = nc.dram_tensor("z1t2", [128, N], mybir.dt.bfloat16, kind="ExternalInput")
    z2t = nc.dram_tensor("z2t", [128, JSHARD], mybir.dt.bfloat16, kind="ExternalInput")
    biasd = nc.dram_tensor("biasd", [128, JTILES], mybir.dt.float32, kind="ExternalInput")
    b2d = nc.dram_tensor("b2d", [128, JTILES], mybir.dt.float32, kind="ExternalInput")
    accd = nc.dram_tensor("accd", [128, N], mybir.dt.bfloat16, kind="ExternalOutput")

    EXP = mybir.ActivationFunctionType.Exp
    ADD = mybir.AluOpType.add
    MULT = mybir.AluOpType.mult
    sch = _sch_steps()

    with tile.TileContext(nc) as tc:
        with (
            tc.tile_pool(name="const", bufs=1) as cpool,
            tc.tile_pool(name="esc", bufs=3) as epool,
            tc.tile_pool(name="psum", bufs=2, space="PSUM") as ppool,
        ):
            z1_sb = cpool.tile([128, N], mybir.dt.bfloat16)
            z2_sb = cpool.tile([128, JSHARD], mybir.dt.bfloat16)
            bias_sb = cpool.tile([128, JTILES], mybir.dt.float32)
            b2_sb = cpool.tile([128, JTILES], mybir.dt.float32)
            acc_sb = cpool.tile([128, N], mybir.dt.bfloat16)

            # DMA order: what the first matmuls need lands first; the first
            # 512 z1 columns ship alone so matmul 0 starts ~1us sooner
            nc.sync.dma_start(z2_sb[:], z2t[:])
            nc.sync.dma_start(z1_sb[:, 0:512], z1t2[:, 0:512])
            nc.sync.dma_start(z1_sb[:, 512:ICHUNK], z1t2[:, 512:ICHUNK])
            nc.sync.dma_start(bias_sb[:], biasd[:])
            nc.sync.dma_start(b2_sb[:], b2d[:])
            for ic in range(1, NIC):
                sl = slice(ic * ICHUNK, (ic + 1) * ICHUNK)
                nc.sync.dma_start(z1_sb[:, sl], z1t2[:, sl])

            for ic in range(NIC):
                sl = slice(ic * ICHUNK, (ic + 1) * ICHUNK)
                for jt in range(JTILES):
                    lhsT = z2_sb[:, jt * 128 : (jt + 1) * 128]
                    ps = ppool.tile([128, ICHUNK], mybir.dt.float32)
                    for b in range(4):
                        j0 = ic * ICHUNK + b * 512
                        nc.tensor.matmul(
                            ps[:, b * 512 : (b + 1) * 512],
                            lhsT,
                            z1_sb[:, j0 : j0 + 512],
                            start=True,
                            stop=True,
                        )
                    if (ic, jt) in sch:
                        # uint16 out: f32->u16 saturates negatives to 0 =
                        # bf16 +0.0, giving exp underflow for free (i16 would
                        # leave small negatives as NaN-pattern bit garbage)
                        it = epool.tile([128, ICHUNK], mybir.dt.uint16)
                        nc.vector.tensor_scalar(
                            out=it[:], in0=ps[:],
                            scalar1=SCHRA_A, scalar2=b2_sb[:, jt : jt + 1],
                            op0=MULT, op1=ADD,
                        )
                        nc.vector.tensor_tensor(
                            out=acc_sb[:, sl], in0=acc_sb[:, sl],
                            in1=it[:].bitcast(mybir.dt.bfloat16), op=ADD,
                        )
                    elif jt == 0:
                        nc.scalar.activation(
                            acc_sb[:, sl], ps[:], EXP, bias=bias_sb[:, jt : jt + 1]
                        )
                    else:
                        e = epool.tile([128, ICHUNK], mybir.dt.bfloat16)
                        nc.scalar.activation(
                            e[:], ps[:], EXP, bias=bias_sb[:, jt : jt + 1]
                        )
                        nc.vector.tensor_tensor(
                            out=acc_sb[:, sl], in0=acc_sb[:, sl], in1=e[:], op=ADD
                        )
                nc.sync.dma_start(accd[:, sl], acc_sb[:, sl])

    nc.compile()
    if NOLOAD_MM:
        n = _strip_redundant_ldweights(nc)
        assert n > 60, f"ldweights strip removed only {n}"
    return nc


def _get_nc():
    global _NC_CACHE
    if _NC_CACHE is None:
        _NC_CACHE = _build_nc()
    return _NC_CACHE


def _prep_inputs(z1, z2):
    z1 = np.asarray(z1, dtype=np.float32)
    z2 = np.asarray(z2, dtype=np.float32)
    z1t2 = np.ascontiguousarray((2.0 * z1.astype(np.float64)).astype(BF16).T)
    z2b = z2.astype(BF16)
    sq2 = (z2b.astype(np.float64) ** 2).sum(axis=-1)  # from the bf16 values
    bias_full = (CSHIFT - sq2).astype(np.float32)     # [N]
    b2_full = (SCHRA_A * bias_full.astype(np.float64) + SCHRA_B).astype(np.float32)
    in_maps = []
    for c in range(NCORES):
        jsl = slice(c * JSHARD, (c + 1) * JSHARD)
        z2t = np.ascontiguousarray(z2b[jsl].T)  # [128, JSHARD]
        bias = np.ascontiguousarray(
            bias_full[jsl].reshape(JTILES, 128).T  # [128, JTILES]
        )
        b2 = np.ascontiguousarray(b2_full[jsl].reshape(JTILES, 128).T)
        in_maps.append({"z1t2": z1t2, "z2t": z2t, "biasd": bias, "b2d": b2})
    return in_maps


def _finish(z1, z2, res_list):
    rows = np.zeros(N, np.float64)
    for r in res_list:
        rows += np.asarray(r["accd"], np.float64).sum(axis=0)
    z1 = np.asarray(z1, dtype=np.float64)
    z2 = np.asarray(z2, dtype=np.float64)
    sq2 = (z2.astype(BF16).astype(np.float64) ** 2).sum(axis=-1)
    tdiag = 2.0 * (z1 * z2).sum(axis=-1) - sq2
    loss = np.mean(np.log(rows) - CSHIFT - tdiag)
    return np.asarray(loss, dtype=np.float32)


def _ensure_hook_shim():
    """bass_utils imports antenv.axon_hooks whenever tracing is requested
    (e.g. via a BASS_TRACE env var); this image's antenv lacks that module.
    Provide an inert registry so tracing degrades to a warning instead of an
    ImportError.  A previously installed real shim is left untouched."""
    import sys

    try:
        import antenv.axon_hooks  # noqa: F401
    except ImportError:
        import types

        import antenv

        mod = types.ModuleType("antenv.axon_hooks")
        mod._hook = None
        mod.set_axon_ntff_profile_hook = lambda h: setattr(mod, "_hook", h)
        mod.get_axon_ntff_profile_hook = lambda: mod._hook
        sys.modules["antenv.axon_hooks"] = mod
        antenv.axon_hooks = mod


def _run(z1, z2, **spmd_kwargs):
    _ensure_hook_shim()
    from concourse.bass_utils import run_bass_kernel_spmd

    in_maps = _prep_inputs(z1, z2)
    res = run_bass_kernel_spmd(
        _get_nc(), in_maps, core_ids=list(range(NCORES)), **spmd_kwargs
    )
    return _finish(z1, z2, res.results), res


def kernel(z1, z2):
    loss, _ = _run(z1, z2)
    return loss
